# revision 26
# baseline (speedup 1.0000x reference)
"""Trainium2 Bass kernel for SAGAN-style self-attention.

Reference computation (per sample, B=8 samples over 8 cores):
    xf = x.reshape(N=4096, C=64)
    f = xf @ Wf + bf            # [N, 8]
    g = xf @ Wg + bg            # [N, 8]
    h = xf @ Wh + bh            # [N, 64]
    s = g @ f.T                 # [N, N]
    beta = softmax(s, axis=-1)
    out = gamma * (beta @ h) + xf

Device-side layout (per core, sample i):
  - st = s.T computed as [m(part), n(free)] tiles; softmax's sum over m
    rides the beta@h matmul as a 65th "ones" column of h (produced by an
    extra unit column in the augmented Wh; gamma is folded into Wh/bh on
    the host so the epilogue is out = oT/Z + x).
  - exp of the 16.7M logits is split between ScalarE (true exp, bf16
    out) and VectorE (Schraudolph: uint16 bits = 184.665*s + 16251,
    bit-cast to bf16; the systematic ~3% ripple washes out in the
    softmax ratio and the constant factor cancels exactly).
  - score matmuls in bf16, 3-way row-tiled over the 8-deep contraction;
    f^T/g^T replicas at partition offsets 0/32/64 are produced by ONE
    [65,16] combined f|g projection matmul per 512-block plus SBUF->SBUF
    DMA fan-out (cheap DMA replaces expensive DVE/ACT replica copies).
  - beta@h accumulates a whole S-block (32 chunk matmuls) into a single
    PSUM bank (banks 6/7 alternate per S-block): no DVE partial-sum
    adds at all.  Epilogue: ACT copies the [65,512] accumulator to bf16
    SBUF, PE transposes it back into the freed bank as bf16, DVE makes
    one 2x packed copy out, one reciprocal of the Z column and 4
    scalar_tensor_tensor ops fuse 1/Z scaling with the residual add.
  - PSUM: banks 0-5 hold two rotating 3-bank st spans; banks 6-7
    alternate as the per-S-block beta@h accumulator (and, during setup,
    f/g/h production scratch + epilogue transpose space).
"""

import numpy as np

N = 4096
C = 64
D = 8
NCHUNK = 32  # m-chunks of 128
SBLK = 512  # n-block width
NS = N // SBLK  # 8 S-blocks
NCORES = 8

# Schraudolph bf16-bits exp: bits_u16 = trunc(A*s + B)
SCHRA_A = 184.66496280094332  # 128 * log2(e)
SCHRA_B = 16251.0  # 127*128 - 5.5 (balanced spline err) + 0.5 (trunc)

# fraction of exp spans on ScalarE (rest on VectorE via Schraudolph).
# consecutive spans should alternate engines (they run concurrently in the
# two-span PSUM ring); the accumulator pattern below does that naturally.
AFRAC = 0.57

# spans of m-chunks per S-block: 10x3 + 1x2 (s=0 ramps 1,2 so the first
# exps need no f/g partition replicas)
SPANS = []  # (s, chunk_start, width)
for _s in range(NS):
    sizes = ([1, 2] + [3] * 9 + [2]) if _s == 0 else ([3] * 10 + [2])
    _c = 0
    for _w in sizes:
        SPANS.append((_s, _c, _w))
        _c += _w

_cache = {}


def _build_nc():
    import concourse.bacc as bacc
    import concourse.tile as tile
    from concourse import mybir

    f32 = mybir.dt.float32
    bf16 = mybir.dt.bfloat16
    u16 = mybir.dt.uint16
    EXP = mybir.ActivationFunctionType.Exp
    MUL = mybir.AluOpType.mult
    ADD = mybir.AluOpType.add

    nc = bacc.Bacc("TRN2", target_bir_lowering=False, debug=False)

    xr_ext = nc.declare_dram_parameter("xr", [128, NCHUNK, C], f32, isOutput=False)
    xTb_ext = nc.declare_dram_parameter("xTb", [C + 1, N], bf16, isOutput=False)
    wfg_ext = nc.declare_dram_parameter("wfg", [C + 1, 2 * D], bf16, isOutput=False)
    whb_ext = nc.declare_dram_parameter("whb", [C + 1, C], bf16, isOutput=False)
    id_ext = nc.declare_dram_parameter("ident", [128, 128], bf16, isOutput=False)
    out_ext = nc.declare_dram_parameter("out", [N, C], f32, isOutput=True)

    # span -> engine assignment (True = ScalarE)
    n_iter = len(SPANS)
    span_on_act = []
    _acc = 0.0
    for _k in range(n_iter):
        _acc += AFRAC
        if _acc >= 1.0:
            span_on_act.append(True)
            _acc -= 1.0
        else:
            span_on_act.append(False)

    with tile.TileContext(nc) as tc:
        with (
            tc.tile_pool(name="singles", bufs=1) as singles,
            tc.tile_pool(name="exp_sb", bufs=24) as exp_pool,
            tc.tile_pool(name="oT_sb", bufs=4) as oT_pool,
            tc.tile_pool(name="tr_sb", bufs=4) as tr_pool,
            tc.tile_pool(name="small", bufs=16) as small,
            tc.tile_pool(name="outsb", bufs=8) as out_pool,
        ):
            # ---- persistent SBUF tensors ----
            x_sb = singles.tile([128, NCHUNK, C], f32)
            wfg_sb = singles.tile([C + 1, 2 * D], bf16)
            xTb_sb = singles.tile([C + 1, N], bf16)
            whb_sb = singles.tile([C + 1, C], bf16)
            id_sb = singles.tile([128, 128], bf16)
            fT_sb = singles.tile([72, N], bf16)
            gT_sb = singles.tile([72, N], bf16)
            h_sb = singles.tile([128, NCHUNK, 128], bf16)
            dummy = singles.tile([128, 1], f32)

            # warm the ACT exp table while input DMAs run
            nc.vector.memset(dummy, 0.0)
            nc.scalar.activation(dummy, dummy, EXP)

            # weights first on the fast HWDGE queues (sync) so the first
            # projection can start ASAP; bulk xT chunked right behind;
            # residual x on the scalar HWDGE queue; replica fan-outs go on
            # the gpsimd (SWDGE) queue
            nc.sync.dma_start(out=wfg_sb, in_=wfg_ext[:])
            nc.scalar.dma_start(out=whb_sb, in_=whb_ext[:])
            nc.scalar.dma_start(out=id_sb, in_=id_ext[:])
            for blk in range(NS):
                nc.sync.dma_start(
                    out=xTb_sb[:, blk * SBLK : (blk + 1) * SBLK],
                    in_=xTb_ext[:, blk * SBLK : (blk + 1) * SBLK],
                )
            nc.scalar.dma_start(out=x_sb, in_=xr_ext[:])

            st_psum_cm = tc.tile_pool(name="st_psum", bufs=1, space="PSUM")
            st_psum = st_psum_cm.__enter__()
            # one tensor spanning all 8 PSUM banks; Tile tracks dependencies
            # at bank granularity.  banks 0-5: st spans; 6-7: o accumulators
            # (and, during setup, f/g/h production scratch)
            big = st_psum.tile([128, 8 * SBLK], f32)

            exp_tiles = [None] * n_iter

            # ---- f|g combined projection: one [65,16] matmul per block,
            #      ACT copy to SBUF (f rows 0-7, g parked at rows 8-15 of
            #      fT_sb), then SBUF->SBUF DMA fan-out to partition
            #      offsets 0/32/64 of fT_sb and gT_sb ----
            for blk in range(NS):
                ps = big[0:16, (6 + blk % 2) * SBLK : (6 + blk % 2) * SBLK + SBLK]
                nc.tensor.matmul(
                    ps,
                    lhsT=wfg_sb[:],
                    rhs=xTb_sb[:, blk * SBLK : (blk + 1) * SBLK],
                    start=True,
                    stop=True,
                )
                nc.scalar.copy(fT_sb[0:16, blk * SBLK : (blk + 1) * SBLK], ps)
                # replica fan-out: per-block for the first two blocks (they
                # gate the first st spans), pairs after
                if blk < 2 or blk % 2 == 1:
                    lo = (blk - 1) * SBLK if (blk >= 2 and blk % 2 == 1) else blk * SBLK
                    q = slice(lo, (blk + 1) * SBLK)
                    nc.gpsimd.dma_start(out=fT_sb[32:40, q], in_=fT_sb[0:8, q])
                    nc.gpsimd.dma_start(out=fT_sb[64:72, q], in_=fT_sb[0:8, q])
                    nc.gpsimd.dma_start(out=gT_sb[0:8, q], in_=fT_sb[8:16, q])
                    nc.gpsimd.dma_start(out=gT_sb[32:40, q], in_=fT_sb[8:16, q])
                    nc.gpsimd.dma_start(out=gT_sb[64:72, q], in_=fT_sb[8:16, q])

            # ---- h production: 4 chunk matmuls into bank-6/7 sub-slots,
            #      then one strided cast moves all four.  col 64 (Z-ones)
            #      via memset; cols 65-127 are zero padding for fast
            #      128-col weight loads ----
            nc.gpsimd.memset(h_sb[:, :, C + 1 : 128], 0.0)
            nc.gpsimd.memset(h_sb[:, :, C : C + 1], 1.0)

            def emit_h(t0):
                for t in range(t0, t0 + 4):
                    hps = big[:, 6 * SBLK + (t % 16) * C :
                              6 * SBLK + (t % 16 + 1) * C]
                    nc.tensor.matmul(
                        hps,
                        lhsT=xTb_sb[:, t * 128 : (t + 1) * 128],
                        rhs=whb_sb[:],
                        start=True,
                        stop=True,
                    )
                s0 = t0 % 16
                blk4 = big[:, 6 * SBLK + s0 * C : 6 * SBLK + (s0 + 4) * C]
                nc.vector.tensor_copy(
                    h_sb[:, t0 : t0 + 4, 0:C],
                    blk4.rearrange("p (b x) -> p b x", b=4),
                )

            # first half of h before the st pipeline (its xTb blocks are
            # already resident); second half after the first two st spans
            # so late xTb DMAs don't stall the PE queue ahead of st(0)
            for t0 in range(0, 16, 4):
                emit_h(t0)

            def emit_st(k):
                s, c0, w = SPANS[k]
                base = (k % 2) * 3 * SBLK
                for j in range(w):
                    mc = c0 + j
                    nc.tensor.matmul(
                        big[:, base + j * SBLK : base + (j + 1) * SBLK],
                        lhsT=fT_sb[32 * j : 32 * j + D, mc * 128 : (mc + 1) * 128],
                        rhs=gT_sb[32 * j : 32 * j + D, s * SBLK : (s + 1) * SBLK],
                        start=True,
                        stop=True,
                        tile_position=(32 * j, 0),
                    )
                expt = exp_pool.tile([128, 3 * SBLK], bf16, tag="exp")
                exp_tiles[k] = expt
                if span_on_act[k]:
                    nc.scalar.activation(
                        expt[:, 0 : w * SBLK], big[:, base : base + w * SBLK], EXP
                    )
                else:
                    nc.vector.tensor_scalar(
                        expt[:, 0 : w * SBLK].bitcast(u16),
                        big[:, base : base + w * SBLK],
                        SCHRA_A,
                        SCHRA_B,
                        MUL,
                        ADD,
                    )

            first_k_of_s = {}
            for _k, (_s, _c0, _w) in enumerate(SPANS):
                first_k_of_s.setdefault(_s, _k)

            def emit_o(k):
                # accumulate the whole S-block into one PSUM bank
                s, c0, w = SPANS[k]
                expt = exp_tiles[k]
                bank = 6 + (s % 2)
                acc = big[:, bank * SBLK : (bank + 1) * SBLK]
                for j in range(w):
                    mc = c0 + j
                    nc.tensor.matmul(
                        acc,
                        lhsT=h_sb[:, mc, :],
                        rhs=expt[:, j * SBLK : (j + 1) * SBLK],
                        start=(k == first_k_of_s[s] and j == 0),
                        stop=(c0 + w == NCHUNK and j == w - 1),
                    )

            def emit_epilogue(s, k):
                bank = 6 + (s % 2)
                acc = big[:, bank * SBLK : (bank + 1) * SBLK]
                # 1) ACT: accumulator -> bf16 SBUF
                oT = oT_pool.tile([C + 1, SBLK], bf16, tag="oT")
                nc.scalar.copy(oT[:], acc[0 : C + 1, :])
                # 2) PE: transpose back into the freed bank (bf16 view;
                #    66-wide slots keep PSUM accesses 4-byte aligned)
                trv = acc.bitcast(bf16)  # [128, 1024] bf16 view of the bank
                for j in range(4):
                    nc.tensor.transpose(
                        trv[:, j * 66 : j * 66 + (C + 1)],
                        in_=oT[:, j * 128 : (j + 1) * 128],
                        identity=id_sb[0 : C + 1, 0 : C + 1],
                    )
                # 3) DVE: one packed 2x copy out of PSUM
                tr = tr_pool.tile([128, 4, 66], bf16, tag="tr")
                nc.vector.tensor_copy(
                    tr[:],
                    trv[:, 0 : 4 * 66].rearrange("p (b x) -> p b x", b=4),
                )
                # 4) 1/Z and fused scale+residual
                rz4 = small.tile([128, 4, 1], f32, tag="rz")
                nc.vector.reciprocal(rz4, tr[:, :, C : C + 1])
                ot = out_pool.tile([128, 4, C], f32, tag="ot")
                for j in range(4):
                    nc.vector.scalar_tensor_tensor(
                        ot[:, j, :], tr[:, j, 0:C], rz4[:, j, :],
                        x_sb[:, s * 4 + j, :],
                        MUL, ADD,
                    )
                row = s * 512
                dview = out_ext[row : row + 512, :].rearrange(
                    "(b p) c -> p b c", p=128)
                nc.sync.dma_start(out=dview, in_=ot)

            # ---- main loop: st spans run 2 ahead of o-matmuls, emitted in
            #      PAIRS (st,st,o*6) to halve the st<->o PE boundaries whose
            #      weight-load turnaround is exposed; epilogues slot in one
            #      span after their S-block ends ----
            emit_st(0)
            emit_st(1)
            for t0 in range(16, NCHUNK, 4):
                emit_h(t0)
            pending_epi = []
            for k in range(0, n_iter, 2):
                for k2 in (k + 2, k + 3):
                    if k2 < n_iter:
                        emit_st(k2)
                if pending_epi:
                    emit_epilogue(*pending_epi.pop())
                for ko in (k, k + 1):
                    if ko >= n_iter:
                        continue
                    emit_o(ko)
                    s, c0, w = SPANS[ko]
                    if c0 + w == NCHUNK:
                        pending_epi.append((s, ko))
            if pending_epi:
                emit_epilogue(*pending_epi.pop())

            st_psum_cm.__exit__(None, None, None)

    nc.finalize()
    return nc


def _get_nc():
    if "nc" not in _cache:
        _cache["nc"] = _build_nc()
    return _cache["nc"]


def make_in_maps(x, kernel_f, kernel_g, kernel_h, bias_f, bias_g, bias_h, gamma):
    from ml_dtypes import bfloat16

    x = np.asarray(x, dtype=np.float32)
    gam = np.float32(np.asarray(gamma).reshape(-1)[0])
    wf_aug = np.concatenate(
        [np.asarray(kernel_f, np.float32).reshape(C, D),
         np.asarray(bias_f, np.float32).reshape(1, D)], axis=0)
    wg_aug = np.concatenate(
        [np.asarray(kernel_g, np.float32).reshape(C, D),
         np.asarray(bias_g, np.float32).reshape(1, D)], axis=0)
    wfg = np.concatenate([wf_aug, wg_aug], axis=1)  # [65, 16]
    wh_aug = np.concatenate(
        [np.asarray(kernel_h, np.float32).reshape(C, C),
         np.asarray(bias_h, np.float32).reshape(1, C)], axis=0) * gam
    whb = wh_aug  # [65, 64]; the Z-ones column of h comes from a memset
    ident = np.eye(128, dtype=np.float32)

    in_maps = []
    for i in range(NCORES):
        xf = x[i].reshape(N, C)
        xr = np.ascontiguousarray(xf.reshape(NCHUNK, 128, C).transpose(1, 0, 2))
        xT_aug = np.concatenate(
            [np.ascontiguousarray(xf.T), np.ones((1, N), np.float32)], axis=0)
        in_maps.append({
            "xr": xr, "xTb": xT_aug.astype(bfloat16),
            "wfg": wfg.astype(bfloat16), "whb": whb.astype(bfloat16),
            "ident": ident.astype(bfloat16),
        })
    return in_maps


def kernel(x, kernel_f, kernel_g, kernel_h, bias_f, bias_g, bias_h, gamma):
    from concourse.bass_utils import run_bass_kernel_spmd

    B, H, W, Cin = x.shape
    assert (B, H, W, Cin) == (8, 64, 64, 64)
    nc = _get_nc()
    in_maps = make_in_maps(x, kernel_f, kernel_g, kernel_h,
                           bias_f, bias_g, bias_h, gamma)
    res = run_bass_kernel_spmd(nc, in_maps, core_ids=list(range(NCORES)))
    out = np.stack([res.results[i]["out"] for i in range(NCORES)], axis=0)
    return out.reshape(B, H, W, Cin).astype(np.float32)


# revision 30
# speedup vs baseline: 1.1872x; 1.1872x over previous
"""Trainium2 Bass kernel for SAGAN-style self-attention.

Reference computation (per sample, B=8 samples over 8 cores):
    xf = x.reshape(N=4096, C=64)
    f = xf @ Wf + bf            # [N, 8]
    g = xf @ Wg + bg            # [N, 8]
    h = xf @ Wh + bh            # [N, 64]
    s = g @ f.T                 # [N, N]
    beta = softmax(s, axis=-1)
    out = gamma * (beta @ h) + xf

Device-side layout (per core, sample i):
  - st = s.T computed as [m(part), n(free)] tiles; softmax's sum over m
    rides the beta@h matmul as a 65th "ones" column of h (produced by an
    extra unit column in the augmented Wh; gamma is folded into Wh/bh on
    the host so the epilogue is out = oT/Z + x).
  - exp of the 16.7M logits is split between ScalarE (true exp, bf16
    out) and VectorE (Schraudolph: uint16 bits = 184.665*s + 16251,
    bit-cast to bf16; the systematic ~3% ripple washes out in the
    softmax ratio and the constant factor cancels exactly).
  - score matmuls in bf16, 3-way row-tiled over the 8-deep contraction;
    f^T/g^T replicas at partition offsets 0/32/64 are produced by ONE
    [65,16] combined f|g projection matmul per 512-block plus SBUF->SBUF
    DMA fan-out (cheap DMA replaces expensive DVE/ACT replica copies).
  - beta@h accumulates a whole S-block (32 chunk matmuls) into a single
    PSUM bank (banks 6/7 alternate per S-block): no DVE partial-sum
    adds at all.  Epilogue: ACT copies the [65,512] accumulator to bf16
    SBUF, PE transposes it back into the freed bank as bf16, DVE makes
    one 2x packed copy out, one reciprocal of the Z column and 4
    scalar_tensor_tensor ops fuse 1/Z scaling with the residual add.
  - PSUM: banks 0-5 hold two rotating 3-bank st spans; banks 6-7
    alternate as the per-S-block beta@h accumulator (and, during setup,
    f/g/h production scratch + epilogue transpose space).
  - PE program emits spans in PAIRS (st,st,o*6): consecutive same-shape
    matmuls hide their LDWEIGHTS, so halving the st<->o boundaries
    removes exposed weight-load turnarounds (walrus ldw-opt is off; a
    weight load otherwise serializes with the matmul stream, and
    --enable-ldw-opt=true hard-crashes the device).
"""

import numpy as np

N = 4096
C = 64
D = 8
NCHUNK = 32  # m-chunks of 128
SBLK = 512  # n-block width
NS = N // SBLK  # 8 S-blocks
NCORES = 8

# Schraudolph bf16-bits exp: bits_u16 = trunc(A*s + B)
SCHRA_A = 184.66496280094332  # 128 * log2(e)
SCHRA_B = 16251.0  # 127*128 - 5.5 (balanced spline err) + 0.5 (trunc)

# fraction of exp spans on ScalarE (rest on VectorE via Schraudolph).
# consecutive spans should alternate engines (they run concurrently in the
# two-span PSUM ring); the accumulator pattern below does that naturally.
AFRAC = 0.57

# spans of m-chunks per S-block: 10x3 + 1x2 (s=0 ramps 1,2 so the first
# exps need no f/g partition replicas)
SPANS = []  # (s, chunk_start, width)
for _s in range(NS):
    sizes = ([1, 2] + [3] * 9 + [2]) if _s == 0 else ([3] * 10 + [2])
    _c = 0
    for _w in sizes:
        SPANS.append((_s, _c, _w))
        _c += _w

_cache = {}


def _build_nc():
    import concourse.bacc as bacc
    import concourse.tile as tile
    from concourse import mybir

    f32 = mybir.dt.float32
    bf16 = mybir.dt.bfloat16
    u16 = mybir.dt.uint16
    EXP = mybir.ActivationFunctionType.Exp
    MUL = mybir.AluOpType.mult
    ADD = mybir.AluOpType.add

    nc = bacc.Bacc("TRN2", target_bir_lowering=False, debug=False)

    xr_ext = nc.declare_dram_parameter("xr", [128, NCHUNK, C], f32, isOutput=False)
    xTb_ext = nc.declare_dram_parameter("xTb", [C + 1, N], bf16, isOutput=False)
    wfg_ext = nc.declare_dram_parameter("wfg", [C + 1, 2 * D], bf16, isOutput=False)
    whb_ext = nc.declare_dram_parameter("whb", [C + 1, C], bf16, isOutput=False)
    id_ext = nc.declare_dram_parameter("ident", [128, 128], bf16, isOutput=False)
    out_ext = nc.declare_dram_parameter("out", [N, C], f32, isOutput=True)

    # span -> engine assignment (True = ScalarE)
    n_iter = len(SPANS)
    span_on_act = []
    _acc = 0.0
    for _k in range(n_iter):
        _acc += AFRAC
        if _acc >= 1.0:
            span_on_act.append(True)
            _acc -= 1.0
        else:
            span_on_act.append(False)

    with tile.TileContext(nc) as tc:
        with (
            tc.tile_pool(name="singles", bufs=1) as singles,
            tc.tile_pool(name="exp_sb", bufs=24) as exp_pool,
            tc.tile_pool(name="oT_sb", bufs=4) as oT_pool,
            tc.tile_pool(name="tr_sb", bufs=4) as tr_pool,
            tc.tile_pool(name="small", bufs=16) as small,
            tc.tile_pool(name="outsb", bufs=8) as out_pool,
        ):
            # ---- persistent SBUF tensors ----
            x_sb = singles.tile([128, NCHUNK, C], f32)
            wfg_sb = singles.tile([C + 1, 2 * D], bf16)
            xTb_sb = singles.tile([C + 1, N], bf16)
            whb_sb = singles.tile([C + 1, C], bf16)
            id_sb = singles.tile([128, 128], bf16)
            fT_sb = singles.tile([72, N], bf16)
            gT_sb = singles.tile([72, N], bf16)
            h_sb = singles.tile([128, NCHUNK, 128], bf16)
            dummy = singles.tile([128, 1], f32)

            # warm the ACT exp table while input DMAs run
            nc.vector.memset(dummy, 0.0)
            nc.scalar.activation(dummy, dummy, EXP)

            # weights first on the fast HWDGE queues (sync) so the first
            # projection can start ASAP; bulk xT chunked right behind;
            # residual x on the scalar HWDGE queue; replica fan-outs go on
            # the gpsimd (SWDGE) queue
            nc.sync.dma_start(out=wfg_sb, in_=wfg_ext[:])
            nc.scalar.dma_start(out=whb_sb, in_=whb_ext[:])
            nc.scalar.dma_start(out=id_sb, in_=id_ext[:])
            for blk in range(NS):
                nc.sync.dma_start(
                    out=xTb_sb[:, blk * SBLK : (blk + 1) * SBLK],
                    in_=xTb_ext[:, blk * SBLK : (blk + 1) * SBLK],
                )
            nc.scalar.dma_start(out=x_sb, in_=xr_ext[:])

            st_psum_cm = tc.tile_pool(name="st_psum", bufs=1, space="PSUM")
            st_psum = st_psum_cm.__enter__()
            # one tensor spanning all 8 PSUM banks; Tile tracks dependencies
            # at bank granularity.  banks 0-5: st spans; 6-7: o accumulators
            # (and, during setup, f/g/h production scratch)
            big = st_psum.tile([128, 8 * SBLK], f32)

            exp_tiles = [None] * n_iter

            # ---- f|g combined projection: one [65,16] matmul per block,
            #      ACT copy to SBUF (f rows 0-7, g parked at rows 8-15 of
            #      fT_sb), then SBUF->SBUF DMA fan-out to partition
            #      offsets 0/32/64 of fT_sb and gT_sb ----
            for blk in range(NS):
                ps = big[0:16, (6 + blk % 2) * SBLK : (6 + blk % 2) * SBLK + SBLK]
                nc.tensor.matmul(
                    ps,
                    lhsT=wfg_sb[:],
                    rhs=xTb_sb[:, blk * SBLK : (blk + 1) * SBLK],
                    start=True,
                    stop=True,
                )
                nc.scalar.copy(fT_sb[0:16, blk * SBLK : (blk + 1) * SBLK], ps)
                if blk % 2 == 1:
                    q = slice((blk - 1) * SBLK, (blk + 1) * SBLK)
                    nc.gpsimd.dma_start(out=fT_sb[32:40, q], in_=fT_sb[0:8, q])
                    nc.gpsimd.dma_start(out=fT_sb[64:72, q], in_=fT_sb[0:8, q])
                    nc.gpsimd.dma_start(out=gT_sb[0:8, q], in_=fT_sb[8:16, q])
                    nc.gpsimd.dma_start(out=gT_sb[32:40, q], in_=fT_sb[8:16, q])
                    nc.gpsimd.dma_start(out=gT_sb[64:72, q], in_=fT_sb[8:16, q])

            # ---- h production: 4 chunk matmuls into bank-6/7 sub-slots,
            #      then one strided cast moves all four.  col 64 (Z-ones)
            #      via memset; cols 65-127 are zero padding for fast
            #      128-col weight loads ----
            nc.gpsimd.memset(h_sb[:, :, C + 1 : 128], 0.0)
            nc.gpsimd.memset(h_sb[:, :, C : C + 1], 1.0)

            def emit_h(t0):
                for t in range(t0, t0 + 4):
                    hps = big[:, 6 * SBLK + (t % 16) * C :
                              6 * SBLK + (t % 16 + 1) * C]
                    nc.tensor.matmul(
                        hps,
                        lhsT=xTb_sb[:, t * 128 : (t + 1) * 128],
                        rhs=whb_sb[:],
                        start=True,
                        stop=True,
                    )
                s0 = t0 % 16
                blk4 = big[:, 6 * SBLK + s0 * C : 6 * SBLK + (s0 + 4) * C]
                nc.vector.tensor_copy(
                    h_sb[:, t0 : t0 + 4, 0:C],
                    blk4.rearrange("p (b x) -> p b x", b=4),
                )

            for t0 in range(0, NCHUNK, 4):
                emit_h(t0)

            def emit_st(k):
                s, c0, w = SPANS[k]
                base = (k % 2) * 3 * SBLK
                for j in range(w):
                    mc = c0 + j
                    nc.tensor.matmul(
                        big[:, base + j * SBLK : base + (j + 1) * SBLK],
                        lhsT=fT_sb[32 * j : 32 * j + D, mc * 128 : (mc + 1) * 128],
                        rhs=gT_sb[32 * j : 32 * j + D, s * SBLK : (s + 1) * SBLK],
                        start=True,
                        stop=True,
                        tile_position=(32 * j, 0),
                    )
                expt = exp_pool.tile([128, 3 * SBLK], bf16, tag="exp")
                exp_tiles[k] = expt
                if span_on_act[k]:
                    nc.scalar.activation(
                        expt[:, 0 : w * SBLK], big[:, base : base + w * SBLK], EXP
                    )
                else:
                    nc.vector.tensor_scalar(
                        expt[:, 0 : w * SBLK].bitcast(u16),
                        big[:, base : base + w * SBLK],
                        SCHRA_A,
                        SCHRA_B,
                        MUL,
                        ADD,
                    )

            first_k_of_s = {}
            for _k, (_s, _c0, _w) in enumerate(SPANS):
                first_k_of_s.setdefault(_s, _k)

            def emit_o(k):
                # accumulate the whole S-block into one PSUM bank
                s, c0, w = SPANS[k]
                expt = exp_tiles[k]
                bank = 6 + (s % 2)
                acc = big[:, bank * SBLK : (bank + 1) * SBLK]
                for j in range(w):
                    mc = c0 + j
                    nc.tensor.matmul(
                        acc,
                        lhsT=h_sb[:, mc, :],
                        rhs=expt[:, j * SBLK : (j + 1) * SBLK],
                        start=(k == first_k_of_s[s] and j == 0),
                        stop=(c0 + w == NCHUNK and j == w - 1),
                    )

            def emit_epilogue(s, k):
                bank = 6 + (s % 2)
                acc = big[:, bank * SBLK : (bank + 1) * SBLK]
                # 1) ACT: accumulator -> bf16 SBUF
                oT = oT_pool.tile([C + 1, SBLK], bf16, tag="oT")
                nc.scalar.copy(oT[:], acc[0 : C + 1, :])
                # 2) PE: transpose back into the freed bank (bf16 view;
                #    66-wide slots keep PSUM accesses 4-byte aligned)
                trv = acc.bitcast(bf16)  # [128, 1024] bf16 view of the bank
                for j in range(4):
                    nc.tensor.transpose(
                        trv[:, j * 66 : j * 66 + (C + 1)],
                        in_=oT[:, j * 128 : (j + 1) * 128],
                        identity=id_sb[0 : C + 1, 0 : C + 1],
                    )
                # 3) DVE: one packed 2x copy out of PSUM
                tr = tr_pool.tile([128, 4, 66], bf16, tag="tr")
                nc.vector.tensor_copy(
                    tr[:],
                    trv[:, 0 : 4 * 66].rearrange("p (b x) -> p b x", b=4),
                )
                # 4) 1/Z and fused scale+residual
                rz4 = small.tile([128, 4, 1], f32, tag="rz")
                nc.vector.reciprocal(rz4, tr[:, :, C : C + 1])
                ot = out_pool.tile([128, 4, C], f32, tag="ot")
                for j in range(4):
                    nc.vector.scalar_tensor_tensor(
                        ot[:, j, :], tr[:, j, 0:C], rz4[:, j, :],
                        x_sb[:, s * 4 + j, :],
                        MUL, ADD,
                    )
                row = s * 512
                dview = out_ext[row : row + 512, :].rearrange(
                    "(b p) c -> p b c", p=128)
                nc.sync.dma_start(out=dview, in_=ot)

            # ---- main loop: st spans run 2 ahead of o-matmuls, emitted in
            #      PAIRS (st,st,o*6) to halve the st<->o PE boundaries whose
            #      weight-load turnaround is exposed; epilogues slot in one
            #      span after their S-block ends ----
            emit_st(0)
            emit_st(1)
            pending_epi = []
            for k in range(0, n_iter, 2):
                for k2 in (k + 2, k + 3):
                    if k2 < n_iter:
                        emit_st(k2)
                if pending_epi:
                    emit_epilogue(*pending_epi.pop())
                for ko in (k, k + 1):
                    if ko >= n_iter:
                        continue
                    emit_o(ko)
                    s, c0, w = SPANS[ko]
                    if c0 + w == NCHUNK:
                        pending_epi.append((s, ko))
            if pending_epi:
                emit_epilogue(*pending_epi.pop())

            st_psum_cm.__exit__(None, None, None)

    nc.finalize()
    return nc


def _get_nc():
    if "nc" not in _cache:
        _cache["nc"] = _build_nc()
    return _cache["nc"]


def make_in_maps(x, kernel_f, kernel_g, kernel_h, bias_f, bias_g, bias_h, gamma):
    from ml_dtypes import bfloat16

    x = np.asarray(x, dtype=np.float32)
    gam = np.float32(np.asarray(gamma).reshape(-1)[0])
    wf_aug = np.concatenate(
        [np.asarray(kernel_f, np.float32).reshape(C, D),
         np.asarray(bias_f, np.float32).reshape(1, D)], axis=0)
    wg_aug = np.concatenate(
        [np.asarray(kernel_g, np.float32).reshape(C, D),
         np.asarray(bias_g, np.float32).reshape(1, D)], axis=0)
    wfg = np.concatenate([wf_aug, wg_aug], axis=1)  # [65, 16]
    wh_aug = np.concatenate(
        [np.asarray(kernel_h, np.float32).reshape(C, C),
         np.asarray(bias_h, np.float32).reshape(1, C)], axis=0) * gam
    whb = wh_aug  # [65, 64]; the Z-ones column of h comes from a memset
    ident = np.eye(128, dtype=np.float32)

    in_maps = []
    for i in range(NCORES):
        xf = x[i].reshape(N, C)
        xr = np.ascontiguousarray(xf.reshape(NCHUNK, 128, C).transpose(1, 0, 2))
        xT_aug = np.concatenate(
            [np.ascontiguousarray(xf.T), np.ones((1, N), np.float32)], axis=0)
        in_maps.append({
            "xr": xr, "xTb": xT_aug.astype(bfloat16),
            "wfg": wfg.astype(bfloat16), "whb": whb.astype(bfloat16),
            "ident": ident.astype(bfloat16),
        })
    return in_maps


def kernel(x, kernel_f, kernel_g, kernel_h, bias_f, bias_g, bias_h, gamma):
    from concourse.bass_utils import run_bass_kernel_spmd

    B, H, W, Cin = x.shape
    assert (B, H, W, Cin) == (8, 64, 64, 64)
    nc = _get_nc()
    in_maps = make_in_maps(x, kernel_f, kernel_g, kernel_h,
                           bias_f, bias_g, bias_h, gamma)
    res = run_bass_kernel_spmd(nc, in_maps, core_ids=list(range(NCORES)))
    out = np.stack([res.results[i]["out"] for i in range(NCORES)], axis=0)
    return out.reshape(B, H, W, Cin).astype(np.float32)


# revision 36
# speedup vs baseline: 1.2524x; 1.0549x over previous
"""Trainium2 Bass kernel for SAGAN-style self-attention.

Reference computation (per sample, B=8 samples over 8 cores):
    xf = x.reshape(N=4096, C=64)
    f = xf @ Wf + bf            # [N, 8]
    g = xf @ Wg + bg            # [N, 8]
    h = xf @ Wh + bh            # [N, 64]
    s = g @ f.T                 # [N, N]
    beta = softmax(s, axis=-1)
    out = gamma * (beta @ h) + xf

Device-side layout (per core, sample i):
  - st = s.T computed as [m(part), n(free)] tiles; softmax's sum over m
    rides the beta@h matmul as a 65th "ones" column of h (produced by an
    extra unit column in the augmented Wh; gamma is folded into Wh/bh on
    the host so the epilogue is out = oT/Z + x).
  - exp of the 16.7M logits is split between ScalarE (true exp, bf16
    out) and VectorE (Schraudolph: uint16 bits = 184.665*s + 16251,
    bit-cast to bf16; the systematic ~3% ripple washes out in the
    softmax ratio and the constant factor cancels exactly).
  - score matmuls in bf16, 3-way row-tiled over the 8-deep contraction;
    f^T/g^T replicas at partition offsets 0/32/64 are produced by ONE
    [65,16] combined f|g projection matmul per 512-block plus SBUF->SBUF
    DMA fan-out (cheap DMA replaces expensive DVE/ACT replica copies).
  - beta@h accumulates a whole S-block (32 chunk matmuls) into a single
    PSUM bank (banks 6/7 alternate per S-block): no DVE partial-sum
    adds at all.  Epilogue: ACT copies the [65,512] accumulator to bf16
    SBUF, PE transposes it back into the freed bank as bf16, DVE makes
    one 2x packed copy out, one reciprocal of the Z column and 4
    scalar_tensor_tensor ops fuse 1/Z scaling with the residual add.
  - PSUM: banks 0-5 hold two rotating 3-bank st spans; banks 6-7
    alternate as the per-S-block beta@h accumulator (and, during setup,
    f/g/h production scratch + epilogue transpose space).
  - PE program emits spans in PAIRS (st,st,o*6): consecutive same-shape
    matmuls hide their LDWEIGHTS, so halving the st<->o boundaries
    removes exposed weight-load turnarounds (walrus ldw-opt is off; a
    weight load otherwise serializes with the matmul stream, and
    --enable-ldw-opt=true hard-crashes the device).
"""

import numpy as np

N = 4096
C = 64
D = 8
NCHUNK = 32  # m-chunks of 128
SBLK = 512  # n-block width
NS = N // SBLK  # 8 S-blocks
NCORES = 8

# Schraudolph bf16-bits exp: bits_u16 = trunc(A*s + B)
SCHRA_A = 184.66496280094332  # 128 * log2(e)
SCHRA_B = 16251.0  # 127*128 - 5.5 (balanced spline err) + 0.5 (trunc)

# fraction of exp spans on ScalarE (rest on VectorE via Schraudolph).
# consecutive spans should alternate engines (they run concurrently in the
# two-span PSUM ring); the accumulator pattern below does that naturally.
AFRAC = 0.57

# spans of m-chunks per S-block: 10x3 + 1x2 (s=0 ramps 1,2 so the first
# exps need no f/g partition replicas)
SPANS = []  # (s, chunk_start, width)
for _s in range(NS):
    sizes = ([1, 2] + [3] * 9 + [2]) if _s == 0 else ([3] * 10 + [2])
    _c = 0
    for _w in sizes:
        SPANS.append((_s, _c, _w))
        _c += _w

_cache = {}


def _build_nc():
    import concourse.bacc as bacc
    import concourse.tile as tile
    from concourse import mybir

    f32 = mybir.dt.float32
    bf16 = mybir.dt.bfloat16
    u16 = mybir.dt.uint16
    EXP = mybir.ActivationFunctionType.Exp
    MUL = mybir.AluOpType.mult
    ADD = mybir.AluOpType.add

    nc = bacc.Bacc("TRN2", target_bir_lowering=False, debug=False)

    xr_ext = nc.declare_dram_parameter("xr", [128, NCHUNK, C], f32, isOutput=False)
    xTb_ext = nc.declare_dram_parameter("xTb", [C + 1, N], bf16, isOutput=False)
    wfg_ext = nc.declare_dram_parameter("wfg", [C + 1, 2 * D], bf16, isOutput=False)
    whb_ext = nc.declare_dram_parameter("whb", [C + 1, C], bf16, isOutput=False)
    id_ext = nc.declare_dram_parameter("ident", [128, 128], bf16, isOutput=False)
    out_ext = nc.declare_dram_parameter("out", [N, C], f32, isOutput=True)

    # span -> engine assignment (True = ScalarE)
    n_iter = len(SPANS)
    span_on_act = []
    _acc = 0.0
    for _k in range(n_iter):
        _acc += AFRAC
        if _acc >= 1.0:
            span_on_act.append(True)
            _acc -= 1.0
        else:
            span_on_act.append(False)

    with tile.TileContext(nc) as tc:
        with (
            tc.tile_pool(name="singles", bufs=1) as singles,
            tc.tile_pool(name="exp_sb", bufs=24) as exp_pool,
            tc.tile_pool(name="oT_sb", bufs=4) as oT_pool,
            tc.tile_pool(name="tr_sb", bufs=4) as tr_pool,
            tc.tile_pool(name="small", bufs=16) as small,
            tc.tile_pool(name="outsb", bufs=8) as out_pool,
        ):
            # ---- persistent SBUF tensors ----
            x_sb = singles.tile([128, NCHUNK, C], f32)
            wfg_sb = singles.tile([C + 1, 2 * D], bf16)
            xTb_sb = singles.tile([C + 1, N], bf16)
            whb_sb = singles.tile([C + 1, C], bf16)
            id_sb = singles.tile([128, 128], bf16)
            fT_sb = singles.tile([72, N], bf16)
            gT_sb = singles.tile([72, N], bf16)
            h_sb = singles.tile([128, NCHUNK, 128], bf16)
            dummy = singles.tile([128, 128], f32)

            # warm the ACT exp table while input DMAs run
            nc.vector.memset(dummy, 0.0)
            nc.scalar.activation(dummy[:, 0:1], dummy[:, 0:1], EXP)

            # weights first on the fast HWDGE queues (sync) so the first
            # projection can start ASAP; bulk xT chunked right behind;
            # residual x on the scalar HWDGE queue; replica fan-outs go on
            # the gpsimd (SWDGE) queue
            nc.sync.dma_start(out=wfg_sb, in_=wfg_ext[:])
            nc.scalar.dma_start(out=whb_sb, in_=whb_ext[:])
            nc.scalar.dma_start(out=id_sb, in_=id_ext[:])
            for blk in range(NS):
                nc.sync.dma_start(
                    out=xTb_sb[:, blk * SBLK : (blk + 1) * SBLK],
                    in_=xTb_ext[:, blk * SBLK : (blk + 1) * SBLK],
                )
            # residual x rides last on sync (first needed ~35us in); the
            # scalar HWDGE queue stays free for the replica fan-outs below
            nc.sync.dma_start(out=x_sb, in_=xr_ext[:])

            st_psum_cm = tc.tile_pool(name="st_psum", bufs=1, space="PSUM")
            st_psum = st_psum_cm.__enter__()
            # one tensor spanning all 8 PSUM banks; Tile tracks dependencies
            # at bank granularity.  banks 0-5: st spans; 6-7: o accumulators
            # (and, during setup, f/g/h production scratch)
            big = st_psum.tile([128, 8 * SBLK], f32)

            # warm the PE's HAM clock gate during the DMA ramp: junk
            # matmuls on bank 7 keep activity up so the first real matmuls
            # run closer to 2.4 GHz than the cold 1.2 GHz
            for _w in range(28):
                nc.tensor.matmul(
                    big[0:1, 7 * SBLK : 7 * SBLK + 128],
                    lhsT=dummy[:, 0:1],
                    rhs=dummy[:, 0:128],
                    start=True,
                    stop=True,
                )

            exp_tiles = [None] * n_iter

            # ---- f|g combined projection: one [65,16] matmul per block,
            #      ACT copy to SBUF (f rows 0-7, g parked at rows 8-15 of
            #      fT_sb), then SBUF->SBUF DMA fan-out to partition
            #      offsets 0/32/64 of fT_sb and gT_sb ----
            for blk in range(NS):
                ps = big[0:16, (6 + blk % 2) * SBLK : (6 + blk % 2) * SBLK + SBLK]
                nc.tensor.matmul(
                    ps,
                    lhsT=wfg_sb[:],
                    rhs=xTb_sb[:, blk * SBLK : (blk + 1) * SBLK],
                    start=True,
                    stop=True,
                )
                nc.scalar.copy(fT_sb[0:16, blk * SBLK : (blk + 1) * SBLK], ps)
                if blk % 2 == 1:
                    # fan-out split across two DMA queues so the last
                    # replica lands ~2x sooner
                    q = slice((blk - 1) * SBLK, (blk + 1) * SBLK)
                    nc.gpsimd.dma_start(out=gT_sb[0:8, q], in_=fT_sb[8:16, q])
                    nc.scalar.dma_start(out=fT_sb[32:40, q], in_=fT_sb[0:8, q])
                    nc.gpsimd.dma_start(out=gT_sb[32:40, q], in_=fT_sb[8:16, q])
                    nc.scalar.dma_start(out=fT_sb[64:72, q], in_=fT_sb[0:8, q])
                    nc.gpsimd.dma_start(out=gT_sb[64:72, q], in_=fT_sb[8:16, q])

            # ---- h production: 4 chunk matmuls into bank-6/7 sub-slots,
            #      then one strided cast moves all four.  col 64 (Z-ones)
            #      via memset; cols 65-127 are zero padding for fast
            #      128-col weight loads ----
            nc.gpsimd.memset(h_sb[:, :, C + 1 : 128], 0.0)
            nc.gpsimd.memset(h_sb[:, :, C : C + 1], 1.0)

            def emit_h(t0):
                for t in range(t0, t0 + 4):
                    hps = big[:, 6 * SBLK + (t % 16) * C :
                              6 * SBLK + (t % 16 + 1) * C]
                    nc.tensor.matmul(
                        hps,
                        lhsT=xTb_sb[:, t * 128 : (t + 1) * 128],
                        rhs=whb_sb[:],
                        start=True,
                        stop=True,
                    )
                s0 = t0 % 16
                blk4 = big[:, 6 * SBLK + s0 * C : 6 * SBLK + (s0 + 4) * C]
                nc.vector.tensor_copy(
                    h_sb[:, t0 : t0 + 4, 0:C],
                    blk4.rearrange("p (b x) -> p b x", b=4),
                )

            for t0 in range(0, NCHUNK, 4):
                emit_h(t0)

            def emit_st(k):
                s, c0, w = SPANS[k]
                base = (k % 2) * 3 * SBLK
                for j in range(w):
                    mc = c0 + j
                    nc.tensor.matmul(
                        big[:, base + j * SBLK : base + (j + 1) * SBLK],
                        lhsT=fT_sb[32 * j : 32 * j + D, mc * 128 : (mc + 1) * 128],
                        rhs=gT_sb[32 * j : 32 * j + D, s * SBLK : (s + 1) * SBLK],
                        start=True,
                        stop=True,
                        tile_position=(32 * j, 0),
                    )
                expt = exp_pool.tile([128, 3 * SBLK], bf16, tag="exp")
                exp_tiles[k] = expt
                if span_on_act[k]:
                    nc.scalar.activation(
                        expt[:, 0 : w * SBLK], big[:, base : base + w * SBLK], EXP
                    )
                else:
                    nc.vector.tensor_scalar(
                        expt[:, 0 : w * SBLK].bitcast(u16),
                        big[:, base : base + w * SBLK],
                        SCHRA_A,
                        SCHRA_B,
                        MUL,
                        ADD,
                    )

            first_k_of_s = {}
            for _k, (_s, _c0, _w) in enumerate(SPANS):
                first_k_of_s.setdefault(_s, _k)

            def emit_o(k):
                # accumulate the whole S-block into one PSUM bank
                s, c0, w = SPANS[k]
                expt = exp_tiles[k]
                bank = 6 + (s % 2)
                acc = big[:, bank * SBLK : (bank + 1) * SBLK]
                for j in range(w):
                    mc = c0 + j
                    nc.tensor.matmul(
                        acc,
                        lhsT=h_sb[:, mc, :],
                        rhs=expt[:, j * SBLK : (j + 1) * SBLK],
                        start=(k == first_k_of_s[s] and j == 0),
                        stop=(c0 + w == NCHUNK and j == w - 1),
                    )

            def emit_epilogue(s, k):
                bank = 6 + (s % 2)
                acc = big[:, bank * SBLK : (bank + 1) * SBLK]
                # 1) ACT: accumulator -> bf16 SBUF
                oT = oT_pool.tile([C + 1, SBLK], bf16, tag="oT")
                nc.scalar.copy(oT[:], acc[0 : C + 1, :])
                # 2) PE: transpose back into the freed bank (bf16 view;
                #    66-wide slots keep PSUM accesses 4-byte aligned)
                trv = acc.bitcast(bf16)  # [128, 1024] bf16 view of the bank
                for j in range(4):
                    nc.tensor.transpose(
                        trv[:, j * 66 : j * 66 + (C + 1)],
                        in_=oT[:, j * 128 : (j + 1) * 128],
                        identity=id_sb[0 : C + 1, 0 : C + 1],
                    )
                # 3) DVE: one packed 2x copy out of PSUM
                tr = tr_pool.tile([128, 4, 66], bf16, tag="tr")
                nc.vector.tensor_copy(
                    tr[:],
                    trv[:, 0 : 4 * 66].rearrange("p (b x) -> p b x", b=4),
                )
                # 4) 1/Z and fused scale+residual
                rz4 = small.tile([128, 4, 1], f32, tag="rz")
                nc.vector.reciprocal(rz4, tr[:, :, C : C + 1])
                ot = out_pool.tile([128, 4, C], f32, tag="ot")
                for j in range(4):
                    nc.vector.scalar_tensor_tensor(
                        ot[:, j, :], tr[:, j, 0:C], rz4[:, j, :],
                        x_sb[:, s * 4 + j, :],
                        MUL, ADD,
                    )
                row = s * 512
                dview = out_ext[row : row + 512, :].rearrange(
                    "(b p) c -> p b c", p=128)
                nc.sync.dma_start(out=dview, in_=ot)

            # ---- main loop: st spans run 2 ahead of o-matmuls, emitted in
            #      PAIRS (st,st,o*6) to halve the st<->o PE boundaries whose
            #      weight-load turnaround is exposed; epilogues slot in one
            #      span after their S-block ends ----
            emit_st(0)
            emit_st(1)
            pending_epi = []
            for k in range(0, n_iter, 2):
                for k2 in (k + 2, k + 3):
                    if k2 < n_iter:
                        emit_st(k2)
                if pending_epi:
                    emit_epilogue(*pending_epi.pop())
                for ko in (k, k + 1):
                    if ko >= n_iter:
                        continue
                    emit_o(ko)
                    s, c0, w = SPANS[ko]
                    if c0 + w == NCHUNK:
                        pending_epi.append((s, ko))
            if pending_epi:
                emit_epilogue(*pending_epi.pop())

            st_psum_cm.__exit__(None, None, None)

    nc.finalize()
    return nc


def _get_nc():
    if "nc" not in _cache:
        _cache["nc"] = _build_nc()
    return _cache["nc"]


def make_in_maps(x, kernel_f, kernel_g, kernel_h, bias_f, bias_g, bias_h, gamma):
    from ml_dtypes import bfloat16

    x = np.asarray(x, dtype=np.float32)
    gam = np.float32(np.asarray(gamma).reshape(-1)[0])
    wf_aug = np.concatenate(
        [np.asarray(kernel_f, np.float32).reshape(C, D),
         np.asarray(bias_f, np.float32).reshape(1, D)], axis=0)
    wg_aug = np.concatenate(
        [np.asarray(kernel_g, np.float32).reshape(C, D),
         np.asarray(bias_g, np.float32).reshape(1, D)], axis=0)
    wfg = np.concatenate([wf_aug, wg_aug], axis=1)  # [65, 16]
    wh_aug = np.concatenate(
        [np.asarray(kernel_h, np.float32).reshape(C, C),
         np.asarray(bias_h, np.float32).reshape(1, C)], axis=0) * gam
    whb = wh_aug  # [65, 64]; the Z-ones column of h comes from a memset
    ident = np.eye(128, dtype=np.float32)

    in_maps = []
    for i in range(NCORES):
        xf = x[i].reshape(N, C)
        xr = np.ascontiguousarray(xf.reshape(NCHUNK, 128, C).transpose(1, 0, 2))
        xT_aug = np.concatenate(
            [np.ascontiguousarray(xf.T), np.ones((1, N), np.float32)], axis=0)
        in_maps.append({
            "xr": xr, "xTb": xT_aug.astype(bfloat16),
            "wfg": wfg.astype(bfloat16), "whb": whb.astype(bfloat16),
            "ident": ident.astype(bfloat16),
        })
    return in_maps


def kernel(x, kernel_f, kernel_g, kernel_h, bias_f, bias_g, bias_h, gamma):
    from concourse.bass_utils import run_bass_kernel_spmd

    B, H, W, Cin = x.shape
    assert (B, H, W, Cin) == (8, 64, 64, 64)
    nc = _get_nc()
    in_maps = make_in_maps(x, kernel_f, kernel_g, kernel_h,
                           bias_f, bias_g, bias_h, gamma)
    res = run_bass_kernel_spmd(nc, in_maps, core_ids=list(range(NCORES)))
    out = np.stack([res.results[i]["out"] for i in range(NCORES)], axis=0)
    return out.reshape(B, H, W, Cin).astype(np.float32)
